# revision 3
# baseline (speedup 1.0000x reference)
"""GroupAttention (LeViT-style) Bass/Tile kernel for 8x Trainium2 NeuronCores.

Reference computation (per batch item b of 16):
  xh = x[b] reshaped [H=8, 64, N=1024]
  qkv[h] = W[h] @ xh[h] + b[h]   (grouped 1x1 conv, 192 out ch per head)
  q,k,v = split(qkv, [32, 32, 128])
  attn = softmax(scale * q^T k, axis=-1)        # [N, N] per head
  o[h] = v @ attn^T                              # [128, N]
  out[b] = BN(proj_w @ relu(concat_h o) + proj_b)

Strategy: pure data-parallel over B (2 batch items per core, no collectives).
Per (b,h): compute S^T = (k^T q) directly in [n,m] layout (no transposes),
exp without max-subtraction (logits are O(1) by construction), row sums via
a bf16 add-tree + ones-vector matmul, normalization applied to the small O
tile instead of the big P matrix. All matmuls in float32r (TF32-like, full
PE rate at N>=256); inputs pre-rounded to f32r on the host so DMA feeds
matmuls directly.
"""
import numpy as np

import concourse.bass as bass
import concourse.bacc as bacc
import concourse.mybir as mybir
import concourse.tile as tile
from concourse.bass_utils import run_bass_kernel_spmd

B, DIM, N = 16, 512, 1024
H, KD, D = 8, 32, 128
CG = DIM // H            # 64 in-channels per head group
NCORES = 8
NB = B // NCORES         # 2 batch items per core
NCH = N // 128           # 8 n-chunks
SCALE = KD ** -0.5
EPS = 1e-5

f32 = mybir.dt.float32
f32r = mybir.dt.float32r
bf16 = mybir.dt.bfloat16


def round_f32r(x: np.ndarray) -> np.ndarray:
    xi = np.ascontiguousarray(x, dtype=np.float32).view(np.uint32)
    return ((xi + np.uint32(0x1000)) & np.uint32(0xFFFFE000)).view(np.float32)


def build_program():
    nc = bacc.Bacc("TRN2", target_bir_lowering=False)

    x_d = nc.declare_dram_parameter("x", [NB, DIM, N], f32r, isOutput=False)
    wqk_d = nc.declare_dram_parameter("wqk", [H, CG + 1, 2 * KD], f32r, isOutput=False)
    wv_d = nc.declare_dram_parameter("wv", [H, CG + 1, D], f32r, isOutput=False)
    pwt_d = nc.declare_dram_parameter("pwt", [H, D, DIM], f32r, isOutput=False)
    psc_d = nc.declare_dram_parameter("psc", [4, 128], f32, isOutput=False)
    pbi_d = nc.declare_dram_parameter("pbi", [4, 128], f32, isOutput=False)
    out_d = nc.declare_dram_parameter("out", [NB, DIM, N], f32, isOutput=True)

    with tile.TileContext(nc) as tc:
        with (
            tc.tile_pool(name="singles", bufs=1) as singles,
            tc.tile_pool(name="xq", bufs=2) as xq,
            tc.tile_pool(name="ptp", bufs=9) as ptp,
            tc.tile_pool(name="trees", bufs=2) as trees,
            tc.tile_pool(name="osb", bufs=2) as osb,
            tc.tile_pool(name="outp", bufs=2) as outp,
            tc.tile_pool(name="ps_s", bufs=2, space="PSUM") as ps_s,
            tc.tile_pool(name="ps_st", bufs=2, space="PSUM") as ps_st,
            tc.tile_pool(name="ps_o", bufs=2, space="PSUM") as ps_o,
        ):
            # --- persistent weights ---
            wqk_sb = singles.tile([CG + 1, H, 2 * KD], f32r)
            nc.sync.dma_start(out=wqk_sb, in_=wqk_d[:].rearrange("h c o -> c h o"))
            wv_sb = singles.tile([CG + 1, H, D], f32r)
            nc.sync.dma_start(out=wv_sb, in_=wv_d[:].rearrange("h c o -> c h o"))
            pwt_sb = singles.tile([D, H, 4, 128], f32r)
            nc.sync.dma_start(
                out=pwt_sb, in_=pwt_d[:].rearrange("h d (o4 o) -> d h o4 o", o4=4)
            )
            psc_sb = singles.tile([128, 4], f32)
            nc.sync.dma_start(out=psc_sb, in_=psc_d[:].rearrange("a p -> p a"))
            pbi_sb = singles.tile([128, 4], f32)
            nc.sync.dma_start(out=pbi_sb, in_=pbi_d[:].rearrange("a p -> p a"))
            ones_bf = singles.tile([128, 1], bf16)
            nc.vector.memset(ones_bf, 1.0)

            for b in range(NB):
                o_sb = osb.tile([D, H, N], f32r, tag="osb")
                for h in range(H):
                    # --- load x group, augmented with a ones row (bias trick) ---
                    xr = xq.tile([CG + 1, N], f32r, tag="xr")
                    nc.sync.dma_start(
                        out=xr[0:CG, :], in_=x_d[b, h * CG : (h + 1) * CG, :]
                    )
                    nc.vector.memset(xr[CG : CG + 1, :].bitcast(f32), 1.0)

                    # --- qkv grouped conv: q,k = wqk^T @ [x;1]  -> [64, N] ---
                    q_sb = xq.tile([KD, N], f32r, tag="q")
                    k_sb = xq.tile([KD, N], f32r, tag="k")
                    for i in range(2):
                        sl = slice(i * 512, (i + 1) * 512)
                        pqk = ps_s.tile([2 * KD, 512], f32, tag="s")
                        nc.tensor.matmul(
                            pqk, wqk_sb[:, h, :], xr[:, sl], start=True, stop=True
                        )
                        nc.vector.tensor_copy(q_sb[:, sl], pqk[0:KD, :])
                        nc.vector.tensor_copy(k_sb[:, sl], pqk[KD : 2 * KD, :])

                    # --- v^T tiles: [n_chunk, d] = x_aug^T @ wv ---
                    vt_sb = xq.tile([128, NCH, D], f32r, tag="vt")
                    for j in range(NCH):
                        pv = ps_s.tile([128, D], f32, tag="s")
                        nc.tensor.matmul(
                            pv,
                            xr[:, j * 128 : (j + 1) * 128],
                            wv_sb[:, h, :],
                            start=True,
                            stop=True,
                        )
                        nc.vector.tensor_copy(vt_sb[:, j, :], pv)

                    # --- S^T = k^T q per n-chunk; exp -> P^T (f32r) ---
                    pts = []
                    for j in range(NCH):
                        pst = ps_st.tile([128, N], f32, tag="st")
                        for i in range(2):
                            sl = slice(i * 512, (i + 1) * 512)
                            nc.tensor.matmul(
                                pst[:, sl],
                                k_sb[:, j * 128 : (j + 1) * 128],
                                q_sb[:, sl],
                                start=True,
                                stop=True,
                            )
                        pt = ptp.tile([128, N], f32r, tag="pt")
                        nc.scalar.activation(pt, pst, mybir.ActivationFunctionType.Exp)
                        pts.append(pt)

                    # --- row sums: bf16 add-tree over n-chunks, then ones-matmul ---
                    t01 = trees.tile([128, N], bf16, tag="t0")
                    t23 = trees.tile([128, N], bf16, tag="t1")
                    t45 = trees.tile([128, N], bf16, tag="t2")
                    t67 = trees.tile([128, N], bf16, tag="t3")
                    nc.gpsimd.tensor_add(t01, pts[0].bitcast(f32), pts[1].bitcast(f32))
                    nc.gpsimd.tensor_add(t23, pts[2].bitcast(f32), pts[3].bitcast(f32))
                    nc.vector.tensor_add(t45, pts[4].bitcast(f32), pts[5].bitcast(f32))
                    nc.vector.tensor_add(t67, pts[6].bitcast(f32), pts[7].bitcast(f32))
                    u0 = trees.tile([128, N], bf16, tag="u0")
                    u1 = trees.tile([128, N], bf16, tag="u1")
                    nc.gpsimd.tensor_add(u0, t01, t23)
                    nc.vector.tensor_add(u1, t45, t67)
                    rsum = trees.tile([128, N], bf16, tag="rs")
                    nc.gpsimd.tensor_add(rsum, u0, u1)

                    rc = trees.tile([1, N], f32, tag="rc")
                    for i in range(2):
                        sl = slice(i * 512, (i + 1) * 512)
                        prs = ps_s.tile([1, 512], f32, tag="s")
                        nc.tensor.matmul(prs, ones_bf, rsum[:, sl], start=True, stop=True)
                        nc.vector.reciprocal(rc[:, sl], prs)
                    rcb = trees.tile([128, N], f32, tag="rcb")
                    nc.gpsimd.partition_broadcast(rcb, rc)

                    # --- O = v @ P (accumulate over n-chunks) -> [d, m] ---
                    po_a = ps_o.tile([D, 512], f32, tag="o")
                    po_b = ps_o.tile([D, 512], f32, tag="o")
                    po = [po_a, po_b]
                    for j in range(NCH):
                        for i in range(2):
                            sl = slice(i * 512, (i + 1) * 512)
                            nc.tensor.matmul(
                                po[i],
                                vt_sb[:, j, :],
                                pts[j][:, sl],
                                start=(j == 0),
                                stop=(j == NCH - 1),
                            )
                    # normalize by row sums, relu, store for proj
                    for i in range(2):
                        sl = slice(i * 512, (i + 1) * 512)
                        tnorm = xq.tile([D, 512], f32, tag="tn")
                        nc.vector.tensor_mul(tnorm, po[i], rcb[:, sl])
                        nc.vector.tensor_scalar_max(o_sb[:, h, sl], tnorm, 0.0)

                # --- proj conv + BN for this batch item ---
                for ocx in range(4):
                    for mx in range(2):
                        msl = slice(mx * 512, (mx + 1) * 512)
                        pp = ps_st.tile([128, 512], f32, tag="st")
                        for h in range(H):
                            nc.tensor.matmul(
                                pp,
                                pwt_sb[:, h, ocx, :],
                                o_sb[:, h, msl],
                                start=(h == 0),
                                stop=(h == H - 1),
                            )
                        ot = outp.tile([128, 512], f32, tag="ot")
                        nc.vector.tensor_scalar(
                            ot,
                            pp,
                            psc_sb[:, ocx : ocx + 1],
                            pbi_sb[:, ocx : ocx + 1],
                            op0=mybir.AluOpType.mult,
                            op1=mybir.AluOpType.add,
                        )
                        nc.sync.dma_start(
                            out=out_d[b, ocx * 128 : (ocx + 1) * 128, msl], in_=ot
                        )

    nc.compile()
    return nc


_NC = None


def _get_nc():
    global _NC
    if _NC is None:
        _NC = build_program()
    return _NC


def prepare_inputs(x, qkv_w, qkv_b, proj_w, proj_b, bn_gamma, bn_beta, bn_mean, bn_var):
    """Fold scales/biases host-side and round matmul operands to f32r."""
    x = np.asarray(x, dtype=np.float32)
    qkv_w = np.asarray(qkv_w, dtype=np.float32)
    qkv_b = np.asarray(qkv_b, dtype=np.float32)
    proj_w = np.asarray(proj_w, dtype=np.float32)
    proj_b = np.asarray(proj_b, dtype=np.float32)

    # wqk[h, c, o]: o in [0,64) = q (pre-scaled) | k; row c=64 is the bias.
    wqk = np.empty((H, CG + 1, 2 * KD), dtype=np.float32)
    wqk[:, :CG, :KD] = qkv_w[:, :KD, :].transpose(0, 2, 1) * SCALE
    wqk[:, :CG, KD:] = qkv_w[:, KD : 2 * KD, :].transpose(0, 2, 1)
    wqk[:, CG, :KD] = qkv_b[:, :KD] * SCALE
    wqk[:, CG, KD:] = qkv_b[:, KD : 2 * KD]

    wv = np.empty((H, CG + 1, D), dtype=np.float32)
    wv[:, :CG, :] = qkv_w[:, 2 * KD :, :].transpose(0, 2, 1)
    wv[:, CG, :] = qkv_b[:, 2 * KD :]

    # pwt[h, d, oc] = proj_w[oc, h*128+d]
    pwt = proj_w.T.reshape(H, D, DIM).copy()

    inv = np.asarray(bn_gamma, np.float32) / np.sqrt(np.asarray(bn_var, np.float32) + EPS)
    pscale = inv.reshape(4, 128)
    pbias = (proj_b * inv + np.asarray(bn_beta, np.float32)
             - np.asarray(bn_mean, np.float32) * inv).reshape(4, 128)

    base = {
        "wqk": round_f32r(wqk),
        "wv": round_f32r(wv),
        "pwt": round_f32r(pwt),
        "psc": pscale,
        "pbi": pbias,
    }
    in_maps = []
    for c in range(NCORES):
        m = dict(base)
        m["x"] = round_f32r(x[c * NB : (c + 1) * NB])
        in_maps.append(m)
    return in_maps


def run(in_maps, trace=False):
    nc = _get_nc()
    res = run_bass_kernel_spmd(nc, in_maps, list(range(NCORES)), trace=trace)
    out = np.concatenate([res.results[i]["out"] for i in range(NCORES)], axis=0)
    return out, res


def kernel(**inputs):
    in_maps = prepare_inputs(**inputs)
    out, _ = run(in_maps)
    return out


# revision 6
# speedup vs baseline: 1.0761x; 1.0761x over previous
"""GroupAttention (LeViT-style) Bass/Tile kernel for 8x Trainium2 NeuronCores.

Reference computation (per batch item b of 16):
  xh = x[b] reshaped [H=8, 64, N=1024]
  qkv[h] = W[h] @ xh[h] + b[h]   (grouped 1x1 conv, 192 out ch per head)
  q,k,v = split(qkv, [32, 32, 128])
  attn = softmax(scale * q^T k, axis=-1)        # [N, N] per head
  o[h] = v @ attn^T                              # [128, N]
  out[b] = BN(proj_w @ relu(concat_h o) + proj_b)

Strategy: pure data-parallel over B (2 batch items per core, no collectives).
Per (b,h): compute S^T = (k^T q) directly in [n,m] layout (no transposes),
exp without max-subtraction (logits are O(1) by construction), row sums via
a bf16 add-tree + ones-vector matmul, normalization applied to the small O
tile instead of the big P matrix. All matmuls in float32r (TF32-like, full
PE rate at N>=256); inputs pre-rounded to f32r on the host so DMA feeds
matmuls directly.
"""
import os
import numpy as np

import concourse.bass as bass
import concourse.bacc as bacc
import concourse.mybir as mybir
import concourse.tile as tile
from concourse.bass_utils import run_bass_kernel_spmd

B, DIM, N = 16, 512, 1024
H, KD, D = 8, 32, 128
CG = DIM // H            # 64 in-channels per head group
NCORES = 8
NB = B // NCORES         # 2 batch items per core
NCH = N // 128           # 8 n-chunks
SCALE = KD ** -0.5
EPS = 1e-5

f32 = mybir.dt.float32
f32r = mybir.dt.float32r
bf16 = mybir.dt.bfloat16


def round_f32r(x: np.ndarray) -> np.ndarray:
    xi = np.ascontiguousarray(x, dtype=np.float32).view(np.uint32)
    return ((xi + np.uint32(0x1000)) & np.uint32(0xFFFFE000)).view(np.float32)


def build_program():
    nc = bacc.Bacc("TRN2", target_bir_lowering=False)

    x_d = nc.declare_dram_parameter("x", [NB, DIM, N], f32r, isOutput=False)
    wqk_d = nc.declare_dram_parameter("wqk", [H, CG + 1, 2 * KD], f32r, isOutput=False)
    wv_d = nc.declare_dram_parameter("wv", [H, CG + 1, D], f32r, isOutput=False)
    pwt_d = nc.declare_dram_parameter("pwt", [H, D, DIM], f32r, isOutput=False)
    psc_d = nc.declare_dram_parameter("psc", [4, 128], f32, isOutput=False)
    pbi_d = nc.declare_dram_parameter("pbi", [4, 128], f32, isOutput=False)
    out_d = nc.declare_dram_parameter("out", [NB, DIM, N], f32, isOutput=True)

    with tile.TileContext(nc) as tc:
        with (
            tc.tile_pool(name="singles", bufs=1) as singles,
            tc.tile_pool(name="xq", bufs=int(os.environ.get("XQ_BUFS", 2))) as xq,
            tc.tile_pool(name="ptp", bufs=int(os.environ.get("PTP_BUFS", 9))) as ptp,
            tc.tile_pool(name="trees", bufs=int(os.environ.get("TREE_BUFS", 2))) as trees,
            tc.tile_pool(name="osb", bufs=int(os.environ.get("OSB_BUFS", 2))) as osb,
            tc.tile_pool(name="outp", bufs=2) as outp,
            tc.tile_pool(name="ps_s", bufs=int(os.environ.get("PSS_BUFS", 2)), space="PSUM") as ps_s,
            tc.tile_pool(name="ps_st", bufs=2, space="PSUM") as ps_st,
            tc.tile_pool(name="ps_o", bufs=int(os.environ.get("PSO_BUFS", 2)), space="PSUM") as ps_o,
        ):
            # --- persistent weights ---
            wqk_sb = singles.tile([CG + 1, H, 2 * KD], f32r)
            nc.sync.dma_start(out=wqk_sb, in_=wqk_d[:].rearrange("h c o -> c h o"))
            wv_sb = singles.tile([CG + 1, H, D], f32r)
            nc.sync.dma_start(out=wv_sb, in_=wv_d[:].rearrange("h c o -> c h o"))
            pwt_sb = singles.tile([D, H, 4, 128], f32r)
            nc.sync.dma_start(
                out=pwt_sb, in_=pwt_d[:].rearrange("h d (o4 o) -> d h o4 o", o4=4)
            )
            psc_sb = singles.tile([128, 4], f32)
            nc.sync.dma_start(out=psc_sb, in_=psc_d[:].rearrange("a p -> p a"))
            pbi_sb = singles.tile([128, 4], f32)
            nc.sync.dma_start(out=pbi_sb, in_=pbi_d[:].rearrange("a p -> p a"))
            ones_r = singles.tile([128, 1], f32r)
            nc.vector.memset(ones_r.bitcast(f32), 1.0)

            for b in range(NB):
                o_sb = osb.tile([D, H, N], f32r, tag="osb")
                for h in range(H):
                    # --- load x group, augmented with a ones row (bias trick) ---
                    xr = xq.tile([CG + 1, N], f32r, tag="xr")
                    nc.sync.dma_start(
                        out=xr[0:CG, :], in_=x_d[b, h * CG : (h + 1) * CG, :]
                    )
                    nc.vector.memset(xr[CG : CG + 1, :].bitcast(f32), 1.0)

                    # --- qkv grouped conv: q,k = wqk^T @ [x;1]  -> [64, N] ---
                    q_sb = xq.tile([KD, N], f32r, tag="q")
                    k_sb = xq.tile([KD, N], f32r, tag="k")
                    for i in range(2):
                        sl = slice(i * 512, (i + 1) * 512)
                        pqk = ps_s.tile([2 * KD, 512], f32, tag="s")
                        nc.tensor.matmul(
                            pqk, wqk_sb[:, h, :], xr[:, sl], start=True, stop=True
                        )
                        nc.vector.tensor_copy(q_sb[:, sl], pqk[0:KD, :])
                        nc.vector.tensor_copy(k_sb[:, sl], pqk[KD : 2 * KD, :])

                    # --- v^T tiles: [n_chunk, d] = x_aug^T @ wv ---
                    vt_sb = xq.tile([128, NCH, D], f32r, tag="vt")
                    for g in range(2):
                        pv = ps_s.tile([128, 4, D], f32, tag="s")
                        for jj in range(4):
                            j = g * 4 + jj
                            nc.tensor.matmul(
                                pv[:, jj, :],
                                xr[:, j * 128 : (j + 1) * 128],
                                wv_sb[:, h, :],
                                start=True,
                                stop=True,
                            )
                        nc.vector.tensor_copy(vt_sb[:, g * 4 : (g + 1) * 4, :], pv)

                    # --- S^T = k^T q per n-chunk; exp -> P^T (f32r) ---
                    pts = []
                    for j in range(NCH):
                        pst = ps_st.tile([128, N], f32, tag="st")
                        for i in range(2):
                            sl = slice(i * 512, (i + 1) * 512)
                            nc.tensor.matmul(
                                pst[:, sl],
                                k_sb[:, j * 128 : (j + 1) * 128],
                                q_sb[:, sl],
                                start=True,
                                stop=True,
                            )
                        pt = ptp.tile([128, N], f32r, tag="pt")
                        nc.scalar.activation(pt, pst, mybir.ActivationFunctionType.Exp)
                        pts.append(pt)

                    # --- row sums: ones^T @ P accumulated over n-chunks on PE ---
                    rc = trees.tile([1, N], f32, tag="rc")
                    for i in range(2):
                        sl = slice(i * 512, (i + 1) * 512)
                        prs = ps_s.tile([1, 512], f32, tag="s")
                        for j in range(NCH):
                            nc.tensor.matmul(prs, ones_r, pts[j][:, sl],
                                             start=(j == 0), stop=(j == NCH - 1))
                        nc.vector.reciprocal(rc[:, sl], prs)
                    rcb = trees.tile([128, N], f32, tag="rcb")
                    nc.gpsimd.partition_broadcast(rcb, rc)

                    # --- O = v @ P (accumulate over n-chunks) -> [d, m] ---
                    po_a = ps_o.tile([D, 512], f32, tag="o")
                    po_b = ps_o.tile([D, 512], f32, tag="o")
                    po = [po_a, po_b]
                    for j in range(NCH):
                        for i in range(2):
                            sl = slice(i * 512, (i + 1) * 512)
                            nc.tensor.matmul(
                                po[i],
                                vt_sb[:, j, :],
                                pts[j][:, sl],
                                start=(j == 0),
                                stop=(j == NCH - 1),
                            )
                    # normalize by row sums, relu, store for proj
                    for i in range(2):
                        sl = slice(i * 512, (i + 1) * 512)
                        tnorm = xq.tile([D, 512], f32, tag="tn")
                        nc.vector.tensor_mul(tnorm, po[i], rcb[:, sl])
                        nc.vector.tensor_scalar_max(o_sb[:, h, sl], tnorm, 0.0)

                # --- proj conv + BN for this batch item ---
                for ocx in range(4):
                    for mx in range(2):
                        msl = slice(mx * 512, (mx + 1) * 512)
                        pp = ps_st.tile([128, 512], f32, tag="st")
                        for h in range(H):
                            nc.tensor.matmul(
                                pp,
                                pwt_sb[:, h, ocx, :],
                                o_sb[:, h, msl],
                                start=(h == 0),
                                stop=(h == H - 1),
                            )
                        ot = outp.tile([128, 512], f32, tag="ot")
                        nc.vector.tensor_scalar(
                            ot,
                            pp,
                            psc_sb[:, ocx : ocx + 1],
                            pbi_sb[:, ocx : ocx + 1],
                            op0=mybir.AluOpType.mult,
                            op1=mybir.AluOpType.add,
                        )
                        nc.sync.dma_start(
                            out=out_d[b, ocx * 128 : (ocx + 1) * 128, msl], in_=ot
                        )

    nc.compile()
    return nc


_NC = None


def _get_nc():
    global _NC
    if _NC is None:
        _NC = build_program()
    return _NC


def prepare_inputs(x, qkv_w, qkv_b, proj_w, proj_b, bn_gamma, bn_beta, bn_mean, bn_var):
    """Fold scales/biases host-side and round matmul operands to f32r."""
    x = np.asarray(x, dtype=np.float32)
    qkv_w = np.asarray(qkv_w, dtype=np.float32)
    qkv_b = np.asarray(qkv_b, dtype=np.float32)
    proj_w = np.asarray(proj_w, dtype=np.float32)
    proj_b = np.asarray(proj_b, dtype=np.float32)

    # wqk[h, c, o]: o in [0,64) = q (pre-scaled) | k; row c=64 is the bias.
    wqk = np.empty((H, CG + 1, 2 * KD), dtype=np.float32)
    wqk[:, :CG, :KD] = qkv_w[:, :KD, :].transpose(0, 2, 1) * SCALE
    wqk[:, :CG, KD:] = qkv_w[:, KD : 2 * KD, :].transpose(0, 2, 1)
    wqk[:, CG, :KD] = qkv_b[:, :KD] * SCALE
    wqk[:, CG, KD:] = qkv_b[:, KD : 2 * KD]

    wv = np.empty((H, CG + 1, D), dtype=np.float32)
    wv[:, :CG, :] = qkv_w[:, 2 * KD :, :].transpose(0, 2, 1)
    wv[:, CG, :] = qkv_b[:, 2 * KD :]

    # pwt[h, d, oc] = proj_w[oc, h*128+d]
    pwt = proj_w.T.reshape(H, D, DIM).copy()

    inv = np.asarray(bn_gamma, np.float32) / np.sqrt(np.asarray(bn_var, np.float32) + EPS)
    pscale = inv.reshape(4, 128)
    pbias = (proj_b * inv + np.asarray(bn_beta, np.float32)
             - np.asarray(bn_mean, np.float32) * inv).reshape(4, 128)

    base = {
        "wqk": round_f32r(wqk),
        "wv": round_f32r(wv),
        "pwt": round_f32r(pwt),
        "psc": pscale,
        "pbi": pbias,
    }
    in_maps = []
    for c in range(NCORES):
        m = dict(base)
        m["x"] = round_f32r(x[c * NB : (c + 1) * NB])
        in_maps.append(m)
    return in_maps


def run(in_maps, trace=False):
    nc = _get_nc()
    res = run_bass_kernel_spmd(nc, in_maps, list(range(NCORES)), trace=trace)
    out = np.concatenate([res.results[i]["out"] for i in range(NCORES)], axis=0)
    return out, res


def kernel(**inputs):
    in_maps = prepare_inputs(**inputs)
    out, _ = run(in_maps)
    return out
